# revision 30
# baseline (speedup 1.0000x reference)
"""PointGNNConv (sum aggregation) on 8 Trainium2 NeuronCores.

Algebraic decomposition: with f_w = [f_w3; f_wx] (3+128 rows),
    msg_e = relu(edge_feat @ f_w + f_b) = relu(u[src_e] + v[dst_e])
    u_j = pos_j @ f_w3 + x_j @ f_wx
    v_i = (delta_i - pos_i) @ f_w3 + f_b

Sharding: dst-range sharding -- core c owns dst in [c*NPC, (c+1)*NPC).

Two NEFFs. Phase A computes per-node u/v (bf16) on each core's node slice:
matmuls are emitted in weight-stationary pairs (one LDWEIGHTS per two
chunks), HAM warm-up matmuls run while input DMAs land, f_b rides the
Identity-activation bias. Between NEFFs the host expands the u table into
a per-edge stream (pure row gather / reordering, no FP).

Phase B (per core): local dsts are RELABELED by descending degree, so each
section of 128 consecutive dsts has near-uniform degree. Section s gets
T[s] = max-degree-in-section column-aligned chunks: slot p of chunk (s, r)
holds the r-th edge of dst (s*128+p); pad slots carry u = -1e9 so
relu(u + v) == 0 exactly. No overflow machinery (padding ~4%). The whole
stream (~130KB/partition) lives in SBUF: a few giant staged DMAs load it
at line rate while compute chases the stages. Per section: DVE adds
v[dst] (broadcast), relu per supergroup (split DVE/GpSimd by load), then
one identity matmul per chunk accumulates the segment-sum in PSUM
[feat, dst]; g-MLP + residual tail fused per supergroup. The T[] profile
is shared across cores (max) so one NEFF serves all 8 cores.

Measured on HW (8 cores): 1340us (SWDGE gather) -> 181us (dense+overflow
selection matmuls) -> 169us (degree-sorted dense) -> this version.
"""
import sys

sys.path.insert(0, "/opt/trn_rl_repo")

import numpy as np
import ml_dtypes

import concourse.bass as bass
import concourse.mybir as mybir
import concourse.tile as tile
from concourse import bacc
from concourse.bass_utils import run_bass_kernel_spmd

BF16 = mybir.dt.bfloat16
F32 = mybir.dt.float32
AF = mybir.ActivationFunctionType
ALU = mybir.AluOpType

NCORES = 8
BF = ml_dtypes.bfloat16
PAD_U = -1.0e9          # pad slots: relu(PAD_U + v) == 0 exactly
STAGE_CHUNKS = 88       # ~2.9 MB per stream-stage DMA
ACCUM_DMA = False       # SWDGE accum DMA rejected by runtime (queue 14 invalid)
RELU_SC_CHUNKS = 0       # scalar relu head-of-line blocks the tail MLP; keep on DVE


class Cfg:
    def __init__(self, n, e, din, dt=512):
        self.N = n
        self.E = e
        self.DIN = din
        self.NPC = n // NCORES          # nodes (dsts) per core
        self.SEC = 128                  # dsts per section
        self.NSEC = -(-self.NPC // self.SEC)
        self.SG_SECS = 4                # sections per supergroup (psum window)
        self.NSG = -(-self.NSEC // self.SG_SECS)
        self.DT = dt                    # free-dim tile for phase A
        self.TS = None                  # per-section chunk count (shared)
        self.SCB = None                 # per-section stream chunk base
        self.NCH = None                 # total chunks
        self.SGPROC = None              # supergroup processing order
        self.STAGES = None              # [(chunk_lo, chunk_hi), ...]


def _dtiles(total, dt):
    return [(i, min(dt, total - i)) for i in range(0, total, dt)]


# ---------------------------------------------------------------- phase A
def build_phase_a(cfg):
    nc = bacc.Bacc(num_devices=NCORES)
    D = cfg.DIN
    xT = nc.dram_tensor("xT", [D, cfg.NPC], BF16, kind="ExternalInput")
    posT = nc.dram_tensor("posT", [3, cfg.NPC], BF16, kind="ExternalInput")
    h_w1 = nc.dram_tensor("h_w1", [D, D], BF16, kind="ExternalInput")
    h_b1 = nc.dram_tensor("h_b1", [D, 1], F32, kind="ExternalInput")
    h_w2 = nc.dram_tensor("h_w2", [D, 3], BF16, kind="ExternalInput")
    h_b2 = nc.dram_tensor("h_b2", [3, 1], F32, kind="ExternalInput")
    f_w3 = nc.dram_tensor("f_w3", [3, D], BF16, kind="ExternalInput")
    f_wx = nc.dram_tensor("f_wx", [D, D], BF16, kind="ExternalInput")
    f_b = nc.dram_tensor("f_b", [D, 1], F32, kind="ExternalInput")
    uT = nc.dram_tensor("uT", [D, cfg.NPC], BF16, kind="ExternalOutput")
    vT = nc.dram_tensor("vT", [D, cfg.NPC], BF16, kind="ExternalOutput")

    tiles = _dtiles(cfg.NPC, cfg.DT)
    with tile.TileContext(nc) as tc:
        with (
            tc.tile_pool(name="consts", bufs=1) as cp,
            tc.tile_pool(name="work", bufs=3) as wp,
            tc.tile_pool(name="psum", bufs=2, space="PSUM") as pp,
            tc.tile_pool(name="psumv", bufs=3, space="PSUM") as pv,
        ):
            # HAM warm-up: keep the PE busy while input DMAs land so the
            # real matmuls run at 2.4 GHz from the start.
            warm = cp.tile([D, cfg.DT], BF16)
            nc.vector.memset(warm[:], 0.0)
            wps = pp.tile([D, cfg.DT], F32, tag="psu")
            for _ in range(7):
                nc.tensor.matmul(out=wps[:], lhsT=warm[:, :D], rhs=warm[:],
                                 start=True, stop=True)

            # loads, most-urgent first
            xT_sb = cp.tile([D, cfg.NPC], BF16)
            half = (cfg.NPC // 2) // cfg.DT * cfg.DT or cfg.NPC
            nc.sync.dma_start(out=xT_sb[:, :half], in_=xT[:, :half])
            fwx_sb = cp.tile([D, D], BF16)
            nc.sync.dma_start(out=fwx_sb[:], in_=f_wx[:])
            fw3_sb = cp.tile([3, D], BF16)
            nc.sync.dma_start(out=fw3_sb[:], in_=f_w3[:])
            posT_sb = cp.tile([3, cfg.NPC], BF16)
            nc.sync.dma_start(out=posT_sb[:], in_=posT[:])
            w1_sb = cp.tile([D, D], BF16)
            nc.sync.dma_start(out=w1_sb[:], in_=h_w1[:])
            if half < cfg.NPC:
                nc.sync.dma_start(out=xT_sb[:, half:], in_=xT[:, half:])
            w2_sb = cp.tile([D, 3], BF16)
            nc.sync.dma_start(out=w2_sb[:], in_=h_w2[:])
            b1_sb = cp.tile([D, 1], F32)
            nc.sync.dma_start(out=b1_sb[:], in_=h_b1[:])
            b2_sb = cp.tile([3, 1], F32)
            nc.sync.dma_start(out=b2_sb[:], in_=h_b2[:])
            fb_sb = cp.tile([D, 1], F32)
            nc.sync.dma_start(out=fb_sb[:], in_=f_b[:])

            # quarter-granular persistent intermediates: tile-level
            # dependency tracking then lets the passes overlap instead of
            # serializing on one big tile
            QT = 4 * cfg.DT
            quarters = _dtiles(cfg.NPC, QT)
            ut_q = [cp.tile([D, qw], BF16, name=f"utq{qi}")
                    for qi, (_, qw) in enumerate(quarters)]
            t1_q = [cp.tile([D, qw], BF16, name=f"t1q{qi}")
                    for qi, (_, qw) in enumerate(quarters)]
            vt_q = [cp.tile([D, qw], BF16, name=f"vtq{qi}")
                    for qi, (_, qw) in enumerate(quarters)]

            # u-pass: u = pos@f_w3 + x@f_wx.  The matmuls stream with the
            # LDWEIGHTS hidden under the previous FD=512 matmul; DVE casts
            # chase the psum buffers; stores are batched per quarter.
            for i, (off, w) in enumerate(tiles):
                sl = slice(off, off + w)
                qi, qo = off // QT, off % QT
                psu = pp.tile([D, cfg.DT], F32, tag="psu")
                nc.tensor.matmul(out=psu[:, :w], lhsT=fwx_sb[:],
                                 rhs=xT_sb[:, sl], start=True, stop=False)
                nc.tensor.matmul(out=psu[:, :w], lhsT=fw3_sb[:],
                                 rhs=posT_sb[:, sl], start=False, stop=True)
                nc.vector.tensor_copy(out=ut_q[qi][:, qo:qo + w],
                                      in_=psu[:, :w])
                if qo + w == quarters[qi][1] or i == len(tiles) - 1:
                    q0 = quarters[qi][0]
                    nc.gpsimd.dma_start(out=uT[:, q0:off + w],
                                        in_=ut_q[qi][:, :off + w - q0])

            # v-pass 1: t1 = relu(x@h_w1 + h_b1) for all chunks (stationary
            # w1 streams; scalar relus chase into quarter tiles)
            for off, w in tiles:
                sl = slice(off, off + w)
                qi, qo = off // QT, off % QT
                ps1 = pp.tile([D, cfg.DT], F32, tag="psu")
                nc.tensor.matmul(out=ps1[:, :w], lhsT=w1_sb[:],
                                 rhs=xT_sb[:, sl], start=True, stop=True)
                nc.scalar.activation(out=t1_q[qi][:, qo:qo + w],
                                     in_=ps1[:, :w],
                                     func=AF.Relu, bias=b1_sb[:])

            # v-pass 2: delta = tanh(t1@h_w2 + h_b2),
            #           v = (delta - pos)@f_w3 + f_b
            for i, (off, w) in enumerate(tiles):
                sl = slice(off, off + w)
                qi, qo = off // QT, off % QT
                ps2 = pv.tile([3, cfg.DT], F32, tag="ps2")
                nc.tensor.matmul(out=ps2[:, :w], lhsT=w2_sb[:],
                                 rhs=t1_q[qi][:, qo:qo + w],
                                 start=True, stop=True)
                dm = wp.tile([3, cfg.DT], BF16, tag="dm")
                nc.scalar.activation(out=dm[:, :w], in_=ps2[:, :w],
                                     func=AF.Tanh, bias=b2_sb[:])
                nc.vector.tensor_tensor(out=dm[:, :w], in0=dm[:, :w],
                                        in1=posT_sb[:, sl], op=ALU.subtract)
                psv = pv.tile([D, cfg.DT], F32, tag="psv")
                nc.tensor.matmul(out=psv[:, :w], lhsT=fw3_sb[:], rhs=dm[:, :w],
                                 start=True, stop=True)
                nc.vector.tensor_scalar_add(out=vt_q[qi][:, qo:qo + w],
                                            in0=psv[:, :w],
                                            scalar1=fb_sb[:])
                if qo + w == quarters[qi][1] or i == len(tiles) - 1:
                    q0 = quarters[qi][0]
                    nc.sync.dma_start(out=vT[:, q0:off + w],
                                      in_=vt_q[qi][:, :off + w - q0])
    nc.finalize()
    return nc


# ---------------------------------------------------------------- phase B
def build_phase_b(cfg):
    nc = bacc.Bacc(num_devices=NCORES)
    D = cfg.DIN
    TS = cfg.TS
    SCB = cfg.SCB
    NSEC = cfg.NSEC
    SGS = cfg.SG_SECS

    xs = nc.dram_tensor("xs", [128, cfg.NCH, D], BF16, kind="ExternalInput")
    vW = nc.dram_tensor("vW", [128, NSEC, D], BF16, kind="ExternalInput")
    xTb = nc.dram_tensor("xTb", [D, cfg.NPC], BF16, kind="ExternalInput")
    gw1 = nc.dram_tensor("gw1", [D, D], BF16, kind="ExternalInput")
    gb1 = nc.dram_tensor("gb1", [D, 1], F32, kind="ExternalInput")
    gw2 = nc.dram_tensor("gw2", [D, D], BF16, kind="ExternalInput")
    gb2 = nc.dram_tensor("gb2", [D, 1], F32, kind="ExternalInput")
    outT = nc.dram_tensor("outT", [D, cfg.NPC], BF16, kind="ExternalOutput")

    ident = nc.inline_tensor(np.eye(128, dtype=BF), name="ident")
    stage_q = ["scalar", "gpsimd", "sync"]
    accum = ACCUM_DMA

    with tile.TileContext(nc) as tc:
        with (
            tc.tile_pool(name="consts", bufs=1) as cp,
            tc.tile_pool(name="cwork", bufs=2) as wp,
            tc.tile_pool(name="psagg", bufs=2, space="PSUM") as pa,
            tc.tile_pool(name="psmlp", bufs=2, space="PSUM") as pm,
        ):
            ident_sb = cp.tile([128, 128], BF16)
            nc.sync.dma_start(out=ident_sb[:], in_=ident[:])
            gw1_sb = cp.tile([D, D], BF16)
            nc.sync.dma_start(out=gw1_sb[:], in_=gw1[:])
            gw2_sb = cp.tile([D, D], BF16)
            nc.sync.dma_start(out=gw2_sb[:], in_=gw2[:])
            gb1_sb = cp.tile([D, 1], F32)
            nc.sync.dma_start(out=gb1_sb[:], in_=gb1[:])
            gb2_sb = cp.tile([D, 1], F32)
            nc.sync.dma_start(out=gb2_sb[:], in_=gb2[:])
            vW_sb = cp.tile([128, NSEC, D], BF16)
            nc.sync.dma_start(out=vW_sb[:], in_=vW[:])
            # edge stream resident in SBUF as one tile PER SUPERGROUP --
            # tile-granular dependency tracking then lets each group's
            # compute start as soon as its own (large) DMA lands.
            sx_sg = {}
            for qi, sg in enumerate(cfg.SGPROC):
                s0 = sg * SGS
                s1 = min(s0 + SGS, NSEC)
                c0 = int(SCB[s0])
                nch = int(SCB[s1 - 1] + TS[s1 - 1]) - c0
                t = cp.tile([128, nch, D], BF16, name=f"sx{sg}")
                q = getattr(nc, stage_q[qi % 3])
                q.dma_start(out=t[:], in_=xs[:, c0:c0 + nch, :])
                sx_sg[sg] = t
            xtb_sb = cp.tile([D, cfg.NPC], BF16)
            nc.sync.dma_start(out=xtb_sb[:], in_=xTb[:])
            out_all = cp.tile([D, cfg.NPC], BF16)

            for sg in cfg.SGPROC:
                s0 = sg * SGS
                s1 = min(s0 + SGS, NSEC)
                secs = s1 - s0
                c0 = int(SCB[s0])
                nch = int(SCB[s1 - 1] + TS[s1 - 1]) - c0
                sx = sx_sg[sg]

                # msg = relu(u + v[dst]) in place, per section (v broadcast
                # across the section's chunks)
                for sv in range(s0, s1):
                    T = int(TS[sv])
                    o = int(SCB[sv]) - c0
                    blk = sx[:, o:o + T, :]
                    nc.vector.tensor_tensor(
                        out=blk, in0=blk,
                        in1=vW_sb[:, sv, None, :].to_broadcast([128, T, D]),
                        op=ALU.add)
                flat = sx[:, :, :].rearrange("p c f -> p (c f)")
                nc.vector.tensor_relu(flat, flat)

                # segment-sum into psum [feat, dst]
                ps = pa.tile([D, SGS * cfg.SEC], F32, tag="psagg")
                for j, sv in enumerate(range(s0, s1)):
                    T = int(TS[sv])
                    o = int(SCB[sv]) - c0
                    osl = slice(j * cfg.SEC, (j + 1) * cfg.SEC)
                    for r in range(T):
                        nc.tensor.matmul(out=ps[:, osl], lhsT=sx[:, o + r, :],
                                         rhs=ident_sb[:], start=(r == 0),
                                         stop=(r == T - 1))
                aggt = wp.tile([D, SGS * cfg.SEC], BF16, tag="aggt")
                nc.scalar.activation(out=aggt[:, :secs * cfg.SEC],
                                     in_=ps[:, :secs * cfg.SEC], func=AF.Copy)

                # fused tail: out = x + relu(relu(agg@g_w1+g_b1)@g_w2+g_b2)
                sgw = min(cfg.NPC, s1 * cfg.SEC) - s0 * cfg.SEC
                for toff in range(0, sgw, 512):
                    w = min(512, sgw - toff)
                    n0 = s0 * cfg.SEC + toff
                    nsl = slice(n0, n0 + w)
                    asl = slice(toff, toff + w)
                    ph1 = pm.tile([D, 512], F32, tag="ph1")
                    nc.tensor.matmul(out=ph1[:, :w], lhsT=gw1_sb[:],
                                     rhs=aggt[:, asl], start=True, stop=True)
                    h1 = wp.tile([D, 512], BF16, tag="h1")
                    nc.scalar.activation(out=h1[:, :w], in_=ph1[:, :w],
                                         func=AF.Relu, bias=gb1_sb[:])
                    ph2 = pm.tile([D, 512], F32, tag="ph2")
                    nc.tensor.matmul(out=ph2[:, :w], lhsT=gw2_sb[:],
                                     rhs=h1[:, :w], start=True, stop=True)
                    h2 = wp.tile([D, 512], BF16, tag="h2")
                    nc.scalar.activation(out=h2[:, :w], in_=ph2[:, :w],
                                         func=AF.Relu, bias=gb2_sb[:])
                    nc.vector.tensor_tensor(out=out_all[:, nsl],
                                            in0=h2[:, :w],
                                            in1=xtb_sb[:, nsl], op=ALU.add)
                    nc.sync.dma_start(out=outT[:, nsl], in_=out_all[:, nsl])
    nc.finalize()
    return nc


# ------------------------------------------------------------ host side
def _preprocess(cfg, edge_index):
    """Sort edges by dst per core; relabel dsts by descending degree;
    column-aligned chunk layout with per-section depth TS (shared across
    cores so one NEFF serves all). Stream chunk order follows the
    supergroup processing order (ramp group first, then largest-first).

    Sets cfg.TS/SCB/NCH/SGPROC/STAGES. Returns per-core dict with:
      idx  [NCH*128] int64  (src node id per slot, -1 pad)
      perm [NPC]     int64  (relabeled id -> original local id)
    """
    src = np.asarray(edge_index[0], dtype=np.int64)
    dst = np.asarray(edge_index[1], dtype=np.int64)
    order = np.argsort(dst, kind="stable")
    src, dst = src[order], dst[order]
    core = dst // cfg.NPC
    bounds = np.searchsorted(core, np.arange(NCORES + 1))

    percore = []
    ts_mat = np.zeros((NCORES, cfg.NSEC), np.int64)
    for c in range(NCORES):
        lo, hi = bounds[c], bounds[c + 1]
        s, d = src[lo:hi], dst[lo:hi] - c * cfg.NPC
        deg = np.bincount(d, minlength=cfg.NPC)
        perm = np.argsort(-deg, kind="stable")
        inv = np.empty(cfg.NPC, np.int64)
        inv[perm] = np.arange(cfg.NPC)
        degpad = np.zeros(cfg.NSEC * cfg.SEC, np.int64)
        degpad[:cfg.NPC] = deg[perm]
        ts_mat[c] = np.maximum(degpad.reshape(cfg.NSEC, cfg.SEC).max(1), 1)
        percore.append((s, d, deg, perm, inv))
    ts = ts_mat.max(0)
    cfg.TS = ts

    # supergroup processing order: smallest first (fast ramp), then the
    # rest largest-first so the big segment-sum tails overlap later work
    sg_ranges = []
    for sg in range(cfg.NSG):
        s0, s1 = sg * cfg.SG_SECS, min((sg + 1) * cfg.SG_SECS, cfg.NSEC)
        sg_ranges.append((sg, s0, s1, int(ts[s0:s1].sum())))
    smallest = min(sg_ranges, key=lambda t: t[3])
    rest = sorted((t for t in sg_ranges if t[0] != smallest[0]),
                  key=lambda t: -t[3])
    proc = [smallest] + rest
    cfg.SGPROC = [t[0] for t in proc]

    scb = np.zeros(cfg.NSEC, np.int64)
    pos = 0
    stages = []
    st_lo = 0
    for _, s0, s1, n in proc:
        for sv in range(s0, s1):
            scb[sv] = pos
            pos += int(ts[sv])
        if pos - st_lo >= STAGE_CHUNKS or len(stages) == 0:
            stages.append((st_lo, pos))
            st_lo = pos
    if st_lo < pos:
        stages.append((st_lo, pos))
    cfg.SCB = scb
    cfg.NCH = pos
    cfg.STAGES = stages

    out = []
    for c in range(NCORES):
        s, d, deg, perm, inv = percore[c]
        first = np.zeros(cfg.NPC, np.int64)
        np.cumsum(deg[:-1], out=first[1:])
        rank = np.arange(len(d)) - first[d]
        k = inv[d]
        chunk = scb[k >> 7] + rank
        slot = chunk * 128 + (k & 127)
        idx = np.full(cfg.NCH * 128, -1, np.int64)
        idx[slot] = s
        out.append({"idx": idx, "perm": perm})
    return out


def _expand_stream(tbl, idx, nch):
    """Gather rows of tbl by idx (PAD_U row for idx<0) -> [128, nch, D]."""
    rows = np.full((len(idx), tbl.shape[1]), PAD_U, dtype=tbl.dtype)
    valid = idx >= 0
    rows[valid] = tbl[idx[valid]]
    return np.ascontiguousarray(
        rows.reshape(nch, 128, -1).transpose(1, 0, 2))


def run(cfg, inputs, trace=False):
    """Full pipeline. inputs: dict as from setup_inputs (numpy)."""
    x = np.asarray(inputs["x"], np.float32)
    pos = np.asarray(inputs["pos"], np.float32)
    edata = _preprocess(cfg, np.asarray(inputs["edge_index"]))

    h_w1 = np.asarray(inputs["h_w1"], np.float32)
    h_b1 = np.asarray(inputs["h_b1"], np.float32)
    h_w2 = np.asarray(inputs["h_w2"], np.float32)
    h_b2 = np.asarray(inputs["h_b2"], np.float32)
    f_w = np.asarray(inputs["f_w"], np.float32)
    f_b = np.asarray(inputs["f_b"], np.float32)
    g_w1 = np.asarray(inputs["g_w1"], np.float32)
    g_b1 = np.asarray(inputs["g_b1"], np.float32)
    g_w2 = np.asarray(inputs["g_w2"], np.float32)
    g_b2 = np.asarray(inputs["g_b2"], np.float32)

    nc_a = build_phase_a(cfg)
    in_a = []
    for c in range(NCORES):
        sl = slice(c * cfg.NPC, (c + 1) * cfg.NPC)
        in_a.append({
            "xT": np.ascontiguousarray(x[sl].T.astype(BF)),
            "posT": np.ascontiguousarray(pos[sl].T.astype(BF)),
            "h_w1": h_w1.astype(BF), "h_b1": h_b1[:, None],
            "h_w2": h_w2.astype(BF), "h_b2": h_b2[:, None],
            "f_w3": f_w[:3].astype(BF), "f_wx": f_w[3:].astype(BF),
            "f_b": f_b[:, None],
        })
    res_a = run_bass_kernel_spmd(nc_a, in_a, core_ids=list(range(NCORES)),
                                 trace=trace)
    # u table node-major over ALL nodes; v tables per-core node-major
    u_nm = np.concatenate(
        [np.ascontiguousarray(np.asarray(r["uT"]).T) for r in res_a.results],
        axis=0)
    v_nms = [np.ascontiguousarray(np.asarray(r["vT"]).T) for r in res_a.results]

    nc_b = build_phase_b(cfg)
    in_b = []
    for c in range(NCORES):
        sl = slice(c * cfg.NPC, (c + 1) * cfg.NPC)
        ed = edata[c]
        perm = ed["perm"]
        v_nm = v_nms[c]
        # vW [128, NSEC, D]: vW[p, s] = v[perm[s*128+p]] (zero-pad past NPC)
        vpad = np.zeros((cfg.NSEC * cfg.SEC, cfg.DIN), dtype=v_nm.dtype)
        vpad[:cfg.NPC] = v_nm[perm]
        vW = np.ascontiguousarray(
            vpad.reshape(cfg.NSEC, 128, cfg.DIN).transpose(1, 0, 2))
        xl = x[sl].astype(BF)
        in_b.append({
            "xs": _expand_stream(u_nm, ed["idx"], cfg.NCH),
            "vW": vW,
            "xTb": np.ascontiguousarray(xl[perm].T),
            "gw1": g_w1.astype(BF), "gb1": g_b1[:, None],
            "gw2": g_w2.astype(BF), "gb2": g_b2[:, None],
        })
    res_b = run_bass_kernel_spmd(nc_b, in_b, core_ids=list(range(NCORES)),
                                 trace=trace)
    out = np.empty((cfg.N, cfg.DIN), np.float32)
    for c in range(NCORES):
        rows = np.asarray(res_b.results[c]["outT"]).T.astype(np.float32)
        blk = out[c * cfg.NPC:(c + 1) * cfg.NPC]
        blk[edata[c]["perm"]] = rows
    return out, (res_a, res_b)


DEFAULT_CFG = Cfg(n=50000, e=500000, din=128)


def kernel(**inputs):
    out, _ = run(DEFAULT_CFG, inputs)
    return out.astype(np.float32)


# revision 32
# speedup vs baseline: 1.1929x; 1.1929x over previous
"""PointGNNConv (sum aggregation) on 8 Trainium2 NeuronCores.

Algebraic decomposition: with f_w = [f_w3; f_wx] (3+128 rows),
    msg_e = relu(edge_feat @ f_w + f_b) = relu(u[src_e] + v[dst_e])
    u_j = pos_j @ f_w3 + x_j @ f_wx
    v_i = (delta_i - pos_i) @ f_w3 + f_b

Sharding: dst-range sharding -- core c owns dst in [c*NPC, (c+1)*NPC).

Two NEFFs. Phase A computes per-node u/v (bf16) on each core's node slice:
matmuls are emitted in weight-stationary pairs (one LDWEIGHTS per two
chunks), HAM warm-up matmuls run while input DMAs land, f_b rides the
Identity-activation bias. Between NEFFs the host expands the u table into
a per-edge stream (pure row gather / reordering, no FP).

Phase B (per core): local dsts are RELABELED by descending degree, so each
section of 128 consecutive dsts has near-uniform degree. Section s gets
T[s] = max-degree-in-section column-aligned chunks: slot p of chunk (s, r)
holds the r-th edge of dst (s*128+p); pad slots carry u = -1e9 so
relu(u + v) == 0 exactly. No overflow machinery (padding ~4%). The whole
stream (~130KB/partition) lives in SBUF: a few giant staged DMAs load it
at line rate while compute chases the stages. Per section: DVE adds
v[dst] (broadcast), relu per supergroup (split DVE/GpSimd by load), then
one identity matmul per chunk accumulates the segment-sum in PSUM
[feat, dst]; g-MLP + residual tail fused per supergroup. The T[] profile
is shared across cores (max) so one NEFF serves all 8 cores.

Measured on HW (8 cores): 1340us (SWDGE gather) -> 181us (dense+overflow
selection matmuls) -> 169us (degree-sorted dense) -> this version.
"""
import sys

sys.path.insert(0, "/opt/trn_rl_repo")

import numpy as np
import ml_dtypes

import concourse.bass as bass
import concourse.mybir as mybir
import concourse.tile as tile
from concourse import bacc
from concourse.bass_utils import run_bass_kernel_spmd

BF16 = mybir.dt.bfloat16
F32 = mybir.dt.float32
AF = mybir.ActivationFunctionType
ALU = mybir.AluOpType

NCORES = 8
BF = ml_dtypes.bfloat16
PAD_U = -1.0e9          # pad slots: relu(PAD_U + v) == 0 exactly
STAGE_CHUNKS = 88       # ~2.9 MB per stream-stage DMA
ACCUM_DMA = False       # SWDGE accum DMA rejected by runtime (queue 14 invalid)
RELU_SC_CHUNKS = 0       # scalar relu head-of-line blocks the tail MLP; keep on DVE


class Cfg:
    def __init__(self, n, e, din, dt=512):
        self.N = n
        self.E = e
        self.DIN = din
        self.NPC = n // NCORES          # nodes (dsts) per core
        self.SEC = 128                  # dsts per section
        self.NSEC = -(-self.NPC // self.SEC)
        self.SG_SECS = 4                # sections per supergroup (psum window)
        self.NSG = -(-self.NSEC // self.SG_SECS)
        self.DT = dt                    # free-dim tile for phase A
        self.TS = None                  # per-section chunk count (shared)
        self.SCB = None                 # per-section stream chunk base
        self.NCH = None                 # total chunks
        self.SGPROC = None              # supergroup processing order
        self.STAGES = None              # [(chunk_lo, chunk_hi), ...]


def _dtiles(total, dt):
    return [(i, min(dt, total - i)) for i in range(0, total, dt)]


# ---------------------------------------------------------------- phase A
def build_phase_a(cfg):
    nc = bacc.Bacc(num_devices=NCORES)
    D = cfg.DIN
    xT = nc.dram_tensor("xT", [D, cfg.NPC], BF16, kind="ExternalInput")
    posT = nc.dram_tensor("posT", [3, cfg.NPC], BF16, kind="ExternalInput")
    h_w1 = nc.dram_tensor("h_w1", [D, D], BF16, kind="ExternalInput")
    h_b1 = nc.dram_tensor("h_b1", [D, 1], F32, kind="ExternalInput")
    h_w2 = nc.dram_tensor("h_w2", [D, 3], BF16, kind="ExternalInput")
    h_b2 = nc.dram_tensor("h_b2", [3, 1], F32, kind="ExternalInput")
    f_w3 = nc.dram_tensor("f_w3", [3, D], BF16, kind="ExternalInput")
    f_wx = nc.dram_tensor("f_wx", [D, D], BF16, kind="ExternalInput")
    f_b = nc.dram_tensor("f_b", [D, 1], F32, kind="ExternalInput")
    uT = nc.dram_tensor("uT", [D, cfg.NPC], BF16, kind="ExternalOutput")
    vT = nc.dram_tensor("vT", [D, cfg.NPC], BF16, kind="ExternalOutput")

    tiles = _dtiles(cfg.NPC, cfg.DT)
    stash = {}
    with tile.TileContext(nc) as tc:
        with (
            tc.tile_pool(name="consts", bufs=1) as cp,
            tc.tile_pool(name="work", bufs=5) as wp,
            tc.tile_pool(name="psum", bufs=2, space="PSUM") as pp,
            tc.tile_pool(name="psumv", bufs=2, space="PSUM") as pv,
        ):
            # HAM warm-up: keep the PE busy while input DMAs land so the
            # real matmuls run at 2.4 GHz from the start.
            warm = cp.tile([D, cfg.DT], BF16)
            nc.vector.memset(warm[:], 0.0)
            wps = pp.tile([D, cfg.DT], F32, tag="psu")
            for _ in range(7):
                nc.tensor.matmul(out=wps[:], lhsT=warm[:, :D], rhs=warm[:],
                                 start=True, stop=True)

            # loads, most-urgent first
            xT_sb = cp.tile([D, cfg.NPC], BF16)
            half = (cfg.NPC // 2) // cfg.DT * cfg.DT or cfg.NPC
            nc.sync.dma_start(out=xT_sb[:, :half], in_=xT[:, :half])
            fwx_sb = cp.tile([D, D], BF16)
            nc.sync.dma_start(out=fwx_sb[:], in_=f_wx[:])
            fw3_sb = cp.tile([3, D], BF16)
            nc.sync.dma_start(out=fw3_sb[:], in_=f_w3[:])
            posT_sb = cp.tile([3, cfg.NPC], BF16)
            nc.sync.dma_start(out=posT_sb[:], in_=posT[:])
            w1_sb = cp.tile([D, D], BF16)
            nc.sync.dma_start(out=w1_sb[:], in_=h_w1[:])
            if half < cfg.NPC:
                nc.sync.dma_start(out=xT_sb[:, half:], in_=xT[:, half:])
            w2_sb = cp.tile([D, 3], BF16)
            nc.sync.dma_start(out=w2_sb[:], in_=h_w2[:])
            b1_sb = cp.tile([D, 1], F32)
            nc.sync.dma_start(out=b1_sb[:], in_=h_b1[:])
            b2_sb = cp.tile([3, 1], F32)
            nc.sync.dma_start(out=b2_sb[:], in_=h_b2[:])
            fb_sb = cp.tile([D, 1], F32)
            nc.sync.dma_start(out=fb_sb[:], in_=f_b[:])

            # u-pass: u = pos@f_w3 + x@f_wx.  Stationary operands
            # alternate between two weights so LDWEIGHTS hides under the
            # FD=512 matmuls; DVE casts chase into quarter tiles whose
            # stores are batched.
            QT = 4 * cfg.DT
            quarters = _dtiles(cfg.NPC, QT)
            ut_q = [cp.tile([D, qw], BF16, name=f"utq{qi}")
                    for qi, (_, qw) in enumerate(quarters)]
            vt_q = [cp.tile([D, qw], BF16, name=f"vtq{qi}")
                    for qi, (_, qw) in enumerate(quarters)]
            for i, (off, w) in enumerate(tiles):
                sl = slice(off, off + w)
                qi, qo = off // QT, off % QT
                psu = pp.tile([D, cfg.DT], F32, tag="psu")
                nc.tensor.matmul(out=psu[:, :w], lhsT=fwx_sb[:],
                                 rhs=xT_sb[:, sl], start=True, stop=False)
                nc.tensor.matmul(out=psu[:, :w], lhsT=fw3_sb[:],
                                 rhs=posT_sb[:, sl], start=False, stop=True)
                nc.vector.tensor_copy(out=ut_q[qi][:, qo:qo + w],
                                      in_=psu[:, :w])
                if qo + w == quarters[qi][1] or i == len(tiles) - 1:
                    q0 = quarters[qi][0]
                    nc.gpsimd.dma_start(out=uT[:, q0:off + w],
                                        in_=ut_q[qi][:, :off + w - q0])

            # v: delta = tanh(relu(x@h_w1+h_b1)@h_w2+h_b2),
            # v = (delta-pos)@f_w3 + f_b.  Software-pipelined emission so
            # no TensorE instruction waits at the head of the queue: stage
            # offsets 0 / -1 / -3 between the three matmul stages.
            n = len(tiles)
            for i in range(n + 3):
                if i < n:
                    off, w = tiles[i]
                    sl = slice(off, off + w)
                    ps1 = pp.tile([D, cfg.DT], F32, tag="ps1")
                    nc.tensor.matmul(out=ps1[:, :w], lhsT=w1_sb[:],
                                     rhs=xT_sb[:, sl], start=True, stop=True)
                    t1 = wp.tile([D, cfg.DT], BF16, tag="t1")
                    nc.scalar.activation(out=t1[:, :w], in_=ps1[:, :w],
                                         func=AF.Relu, bias=b1_sb[:])
                    stash[i] = t1
                j = i - 1
                if 0 <= j < n:
                    off, w = tiles[j]
                    sl = slice(off, off + w)
                    ps2 = pv.tile([3, cfg.DT], F32, tag="ps2")
                    nc.tensor.matmul(out=ps2[:, :w], lhsT=w2_sb[:],
                                     rhs=stash[j][:, :w], start=True, stop=True)
                    dm = wp.tile([3, cfg.DT], BF16, tag="dm")
                    nc.scalar.activation(out=dm[:, :w], in_=ps2[:, :w],
                                         func=AF.Tanh, bias=b2_sb[:])
                    nc.vector.tensor_tensor(out=dm[:, :w], in0=dm[:, :w],
                                            in1=posT_sb[:, sl],
                                            op=ALU.subtract)
                    stash[j] = dm
                k = i - 3
                if 0 <= k < n:
                    off, w = tiles[k]
                    qi, qo = off // QT, off % QT
                    psv = pv.tile([D, cfg.DT], F32, tag="psv")
                    nc.tensor.matmul(out=psv[:, :w], lhsT=fw3_sb[:],
                                     rhs=stash[k][:, :w], start=True, stop=True)
                    nc.vector.tensor_scalar_add(out=vt_q[qi][:, qo:qo + w],
                                                in0=psv[:, :w],
                                                scalar1=fb_sb[:])
                    if qo + w == quarters[qi][1] or k == n - 1:
                        q0 = quarters[qi][0]
                        nc.sync.dma_start(out=vT[:, q0:off + w],
                                          in_=vt_q[qi][:, :off + w - q0])
    nc.finalize()
    return nc


# ---------------------------------------------------------------- phase B
def build_phase_b(cfg):
    nc = bacc.Bacc(num_devices=NCORES)
    D = cfg.DIN
    TS = cfg.TS
    SCB = cfg.SCB
    NSEC = cfg.NSEC
    SGS = cfg.SG_SECS

    xs = nc.dram_tensor("xs", [128, cfg.NCH, D], BF16, kind="ExternalInput")
    vW = nc.dram_tensor("vW", [128, NSEC, D], BF16, kind="ExternalInput")
    xTb = nc.dram_tensor("xTb", [D, cfg.NPC], BF16, kind="ExternalInput")
    gw1 = nc.dram_tensor("gw1", [D, D], BF16, kind="ExternalInput")
    gb1 = nc.dram_tensor("gb1", [D, 1], F32, kind="ExternalInput")
    gw2 = nc.dram_tensor("gw2", [D, D], BF16, kind="ExternalInput")
    gb2 = nc.dram_tensor("gb2", [D, 1], F32, kind="ExternalInput")
    outT = nc.dram_tensor("outT", [D, cfg.NPC], BF16, kind="ExternalOutput")

    ident = nc.inline_tensor(np.eye(128, dtype=BF), name="ident")

    sg_nch = {}
    for sg in cfg.SGPROC:
        s0, s1 = sg * SGS, min((sg + 1) * SGS, NSEC)
        sg_nch[sg] = int(SCB[s1 - 1] + TS[s1 - 1]) - int(SCB[s0])
    maxc = max(sg_nch.values())

    with tile.TileContext(nc) as tc:
        with (
            tc.tile_pool(name="consts", bufs=1) as cp,
            tc.tile_pool(name="stream", bufs=3) as gp,
            tc.tile_pool(name="cwork", bufs=2) as wp,
            tc.tile_pool(name="psagg", bufs=2, space="PSUM") as pa,
            tc.tile_pool(name="psmlp", bufs=2, space="PSUM") as pm,
        ):
            # vW first: the very first adds need it
            vW_sb = cp.tile([128, NSEC, D], BF16)
            nc.sync.dma_start(out=vW_sb[:], in_=vW[:])
            ident_sb = cp.tile([128, 128], BF16)
            nc.sync.dma_start(out=ident_sb[:], in_=ident[:])
            gw1_sb = cp.tile([D, D], BF16)
            nc.sync.dma_start(out=gw1_sb[:], in_=gw1[:])
            gw2_sb = cp.tile([D, D], BF16)
            nc.sync.dma_start(out=gw2_sb[:], in_=gw2[:])
            gb1_sb = cp.tile([D, 1], F32)
            nc.sync.dma_start(out=gb1_sb[:], in_=gb1[:])
            gb2_sb = cp.tile([D, 1], F32)
            nc.sync.dma_start(out=gb2_sb[:], in_=gb2[:])
            xtb_sb = cp.tile([D, cfg.NPC], BF16)
            nc.sync.dma_start(out=xtb_sb[:], in_=xTb[:])

            for qi, sg in enumerate(cfg.SGPROC):
                s0 = sg * SGS
                s1 = min(s0 + SGS, NSEC)
                secs = s1 - s0
                c0 = int(SCB[s0])
                nch = sg_nch[sg]

                ue = gp.tile([128, maxc, D], BF16, tag="ue")
                dma_q = nc.gpsimd if qi % 2 == 0 else nc.scalar
                dma_q.dma_start(out=ue[:, :nch, :], in_=xs[:, c0:c0 + nch, :])

                # msg = relu(u + v[dst]) in place, per section (v broadcast
                # across the section's chunks)
                for sv in range(s0, s1):
                    T = int(TS[sv])
                    o = int(SCB[sv]) - c0
                    blk = ue[:, o:o + T, :]
                    nc.vector.tensor_tensor(
                        out=blk, in0=blk,
                        in1=vW_sb[:, sv, None, :].to_broadcast([128, T, D]),
                        op=ALU.add)
                flat = ue[:, :nch, :].rearrange("p c f -> p (c f)")
                nc.vector.tensor_relu(flat, flat)

                # segment-sum into psum [feat, dst]
                ps = pa.tile([D, SGS * cfg.SEC], F32, tag="psagg")
                for j, sv in enumerate(range(s0, s1)):
                    T = int(TS[sv])
                    o = int(SCB[sv]) - c0
                    osl = slice(j * cfg.SEC, (j + 1) * cfg.SEC)
                    for r in range(T):
                        nc.tensor.matmul(out=ps[:, osl], lhsT=ue[:, o + r, :],
                                         rhs=ident_sb[:], start=(r == 0),
                                         stop=(r == T - 1))
                aggt = wp.tile([D, SGS * cfg.SEC], BF16, tag="aggt")
                nc.scalar.activation(out=aggt[:, :secs * cfg.SEC],
                                     in_=ps[:, :secs * cfg.SEC], func=AF.Copy)

                # fused tail: out = x + relu(relu(agg@g_w1+g_b1)@g_w2+g_b2)
                sgw = min(cfg.NPC, s1 * cfg.SEC) - s0 * cfg.SEC
                for toff in range(0, sgw, 512):
                    w = min(512, sgw - toff)
                    n0 = s0 * cfg.SEC + toff
                    nsl = slice(n0, n0 + w)
                    asl = slice(toff, toff + w)
                    ph1 = pm.tile([D, 512], F32, tag="ph1")
                    nc.tensor.matmul(out=ph1[:, :w], lhsT=gw1_sb[:],
                                     rhs=aggt[:, asl], start=True, stop=True)
                    h1 = wp.tile([D, 512], BF16, tag="h1")
                    nc.scalar.activation(out=h1[:, :w], in_=ph1[:, :w],
                                         func=AF.Relu, bias=gb1_sb[:])
                    ph2 = pm.tile([D, 512], F32, tag="ph2")
                    nc.tensor.matmul(out=ph2[:, :w], lhsT=gw2_sb[:],
                                     rhs=h1[:, :w], start=True, stop=True)
                    h2 = wp.tile([D, 512], BF16, tag="h2")
                    nc.scalar.activation(out=h2[:, :w], in_=ph2[:, :w],
                                         func=AF.Relu, bias=gb2_sb[:])
                    ob = wp.tile([D, 512], BF16, tag="ob")
                    nc.gpsimd.tensor_tensor(out=ob[:, :w], in0=h2[:, :w],
                                            in1=xtb_sb[:, nsl], op=ALU.add)
                    nc.sync.dma_start(out=outT[:, nsl], in_=ob[:, :w])
    nc.finalize()
    return nc


# ------------------------------------------------------------ host side
def _preprocess(cfg, edge_index):
    """Sort edges by dst per core; relabel dsts by descending degree;
    column-aligned chunk layout with per-section depth TS (shared across
    cores so one NEFF serves all). Stream chunk order follows the
    supergroup processing order (ramp group first, then largest-first).

    Sets cfg.TS/SCB/NCH/SGPROC/STAGES. Returns per-core dict with:
      idx  [NCH*128] int64  (src node id per slot, -1 pad)
      perm [NPC]     int64  (relabeled id -> original local id)
    """
    src = np.asarray(edge_index[0], dtype=np.int64)
    dst = np.asarray(edge_index[1], dtype=np.int64)
    order = np.argsort(dst, kind="stable")
    src, dst = src[order], dst[order]
    core = dst // cfg.NPC
    bounds = np.searchsorted(core, np.arange(NCORES + 1))

    percore = []
    ts_mat = np.zeros((NCORES, cfg.NSEC), np.int64)
    for c in range(NCORES):
        lo, hi = bounds[c], bounds[c + 1]
        s, d = src[lo:hi], dst[lo:hi] - c * cfg.NPC
        deg = np.bincount(d, minlength=cfg.NPC)
        perm = np.argsort(-deg, kind="stable")
        inv = np.empty(cfg.NPC, np.int64)
        inv[perm] = np.arange(cfg.NPC)
        degpad = np.zeros(cfg.NSEC * cfg.SEC, np.int64)
        degpad[:cfg.NPC] = deg[perm]
        ts_mat[c] = np.maximum(degpad.reshape(cfg.NSEC, cfg.SEC).max(1), 1)
        percore.append((s, d, deg, perm, inv))
    ts = ts_mat.max(0)
    cfg.TS = ts

    # supergroup processing order: smallest first (fast ramp), then the
    # rest largest-first so the big segment-sum tails overlap later work
    sg_ranges = []
    for sg in range(cfg.NSG):
        s0, s1 = sg * cfg.SG_SECS, min((sg + 1) * cfg.SG_SECS, cfg.NSEC)
        sg_ranges.append((sg, s0, s1, int(ts[s0:s1].sum())))
    by_size = sorted(sg_ranges, key=lambda t: t[3])
    proc = by_size[:2] + sorted(by_size[2:], key=lambda t: -t[3])
    cfg.SGPROC = [t[0] for t in proc]

    scb = np.zeros(cfg.NSEC, np.int64)
    pos = 0
    stages = []
    st_lo = 0
    for _, s0, s1, n in proc:
        for sv in range(s0, s1):
            scb[sv] = pos
            pos += int(ts[sv])
        if pos - st_lo >= STAGE_CHUNKS or len(stages) == 0:
            stages.append((st_lo, pos))
            st_lo = pos
    if st_lo < pos:
        stages.append((st_lo, pos))
    cfg.SCB = scb
    cfg.NCH = pos
    cfg.STAGES = stages

    out = []
    for c in range(NCORES):
        s, d, deg, perm, inv = percore[c]
        first = np.zeros(cfg.NPC, np.int64)
        np.cumsum(deg[:-1], out=first[1:])
        rank = np.arange(len(d)) - first[d]
        k = inv[d]
        chunk = scb[k >> 7] + rank
        slot = chunk * 128 + (k & 127)
        idx = np.full(cfg.NCH * 128, -1, np.int64)
        idx[slot] = s
        out.append({"idx": idx, "perm": perm})
    return out


def _expand_stream(tbl, idx, nch):
    """Gather rows of tbl by idx (PAD_U row for idx<0) -> [128, nch, D]."""
    rows = np.full((len(idx), tbl.shape[1]), PAD_U, dtype=tbl.dtype)
    valid = idx >= 0
    rows[valid] = tbl[idx[valid]]
    return np.ascontiguousarray(
        rows.reshape(nch, 128, -1).transpose(1, 0, 2))


def run(cfg, inputs, trace=False):
    """Full pipeline. inputs: dict as from setup_inputs (numpy)."""
    x = np.asarray(inputs["x"], np.float32)
    pos = np.asarray(inputs["pos"], np.float32)
    edata = _preprocess(cfg, np.asarray(inputs["edge_index"]))

    h_w1 = np.asarray(inputs["h_w1"], np.float32)
    h_b1 = np.asarray(inputs["h_b1"], np.float32)
    h_w2 = np.asarray(inputs["h_w2"], np.float32)
    h_b2 = np.asarray(inputs["h_b2"], np.float32)
    f_w = np.asarray(inputs["f_w"], np.float32)
    f_b = np.asarray(inputs["f_b"], np.float32)
    g_w1 = np.asarray(inputs["g_w1"], np.float32)
    g_b1 = np.asarray(inputs["g_b1"], np.float32)
    g_w2 = np.asarray(inputs["g_w2"], np.float32)
    g_b2 = np.asarray(inputs["g_b2"], np.float32)

    nc_a = build_phase_a(cfg)
    in_a = []
    for c in range(NCORES):
        sl = slice(c * cfg.NPC, (c + 1) * cfg.NPC)
        in_a.append({
            "xT": np.ascontiguousarray(x[sl].T.astype(BF)),
            "posT": np.ascontiguousarray(pos[sl].T.astype(BF)),
            "h_w1": h_w1.astype(BF), "h_b1": h_b1[:, None],
            "h_w2": h_w2.astype(BF), "h_b2": h_b2[:, None],
            "f_w3": f_w[:3].astype(BF), "f_wx": f_w[3:].astype(BF),
            "f_b": f_b[:, None],
        })
    res_a = run_bass_kernel_spmd(nc_a, in_a, core_ids=list(range(NCORES)),
                                 trace=trace)
    # u table node-major over ALL nodes; v tables per-core node-major
    u_nm = np.concatenate(
        [np.ascontiguousarray(np.asarray(r["uT"]).T) for r in res_a.results],
        axis=0)
    v_nms = [np.ascontiguousarray(np.asarray(r["vT"]).T) for r in res_a.results]

    nc_b = build_phase_b(cfg)
    in_b = []
    for c in range(NCORES):
        sl = slice(c * cfg.NPC, (c + 1) * cfg.NPC)
        ed = edata[c]
        perm = ed["perm"]
        v_nm = v_nms[c]
        # vW [128, NSEC, D]: vW[p, s] = v[perm[s*128+p]] (zero-pad past NPC)
        vpad = np.zeros((cfg.NSEC * cfg.SEC, cfg.DIN), dtype=v_nm.dtype)
        vpad[:cfg.NPC] = v_nm[perm]
        vW = np.ascontiguousarray(
            vpad.reshape(cfg.NSEC, 128, cfg.DIN).transpose(1, 0, 2))
        xl = x[sl].astype(BF)
        in_b.append({
            "xs": _expand_stream(u_nm, ed["idx"], cfg.NCH),
            "vW": vW,
            "xTb": np.ascontiguousarray(xl[perm].T),
            "gw1": g_w1.astype(BF), "gb1": g_b1[:, None],
            "gw2": g_w2.astype(BF), "gb2": g_b2[:, None],
        })
    res_b = run_bass_kernel_spmd(nc_b, in_b, core_ids=list(range(NCORES)),
                                 trace=trace)
    out = np.empty((cfg.N, cfg.DIN), np.float32)
    for c in range(NCORES):
        rows = np.asarray(res_b.results[c]["outT"]).T.astype(np.float32)
        blk = out[c * cfg.NPC:(c + 1) * cfg.NPC]
        blk[edata[c]["perm"]] = rows
    return out, (res_a, res_b)


DEFAULT_CFG = Cfg(n=50000, e=500000, din=128)


def kernel(**inputs):
    out, _ = run(DEFAULT_CFG, inputs)
    return out.astype(np.float32)


# revision 34
# speedup vs baseline: 1.3285x; 1.1136x over previous
"""PointGNNConv (sum aggregation) on 8 Trainium2 NeuronCores.

Algebraic decomposition: with f_w = [f_w3; f_wx] (3+128 rows),
    msg_e = relu(edge_feat @ f_w + f_b) = relu(u[src_e] + v[dst_e])
    u_j = pos_j @ f_w3 + x_j @ f_wx
    v_i = (delta_i - pos_i) @ f_w3 + f_b

Sharding: dst-range sharding -- core c owns dst in [c*NPC, (c+1)*NPC).

Two NEFFs. Phase A computes per-node u/v (bf16) on each core's node slice:
matmuls are emitted in weight-stationary pairs (one LDWEIGHTS per two
chunks), HAM warm-up matmuls run while input DMAs land, f_b rides the
Identity-activation bias. Between NEFFs the host expands the u table into
a per-edge stream (pure row gather / reordering, no FP).

Phase B (per core): local dsts are RELABELED by descending degree, so each
section of 128 consecutive dsts has near-uniform degree. Section s gets
T[s] = max-degree-in-section column-aligned chunks: slot p of chunk (s, r)
holds the r-th edge of dst (s*128+p); pad slots carry u = -1e9 so
relu(u + v) == 0 exactly. No overflow machinery (padding ~4%). The whole
stream (~130KB/partition) lives in SBUF: a few giant staged DMAs load it
at line rate while compute chases the stages. Per section: DVE adds
v[dst] (broadcast), relu per supergroup (split DVE/GpSimd by load), then
one identity matmul per chunk accumulates the segment-sum in PSUM
[feat, dst]; g-MLP + residual tail fused per supergroup. The T[] profile
is shared across cores (max) so one NEFF serves all 8 cores.

Measured on HW (8 cores): 1340us (SWDGE gather) -> 181us (dense+overflow
selection matmuls) -> 169us (degree-sorted dense) -> this version.
"""
import sys

sys.path.insert(0, "/opt/trn_rl_repo")

import numpy as np
import ml_dtypes

import concourse.bass as bass
import concourse.mybir as mybir
import concourse.tile as tile
from concourse import bacc
from concourse.bass_utils import run_bass_kernel_spmd

BF16 = mybir.dt.bfloat16
F32 = mybir.dt.float32
AF = mybir.ActivationFunctionType
ALU = mybir.AluOpType

NCORES = 8
BF = ml_dtypes.bfloat16
PAD_U = -1.0e9          # pad slots: relu(PAD_U + v) == 0 exactly
STAGE_CHUNKS = 88       # ~2.9 MB per stream-stage DMA
ACCUM_DMA = False       # SWDGE accum DMA rejected by runtime (queue 14 invalid)
RELU_SC_CHUNKS = 0       # scalar relu head-of-line blocks the tail MLP; keep on DVE


class Cfg:
    def __init__(self, n, e, din, dt=512):
        self.N = n
        self.E = e
        self.DIN = din
        self.NPC = n // NCORES          # nodes (dsts) per core
        self.SEC = 128                  # dsts per section
        self.NSEC = -(-self.NPC // self.SEC)
        self.SG_SECS = 4                # sections per supergroup (psum window)
        self.NSG = -(-self.NSEC // self.SG_SECS)
        self.DT = dt                    # free-dim tile for phase A
        self.TS = None                  # per-section chunk count (shared)
        self.SCB = None                 # per-section stream chunk base
        self.NCH = None                 # total chunks
        self.SGPROC = None              # supergroup processing order
        self.STAGES = None              # [(chunk_lo, chunk_hi), ...]


def _dtiles(total, dt):
    return [(i, min(dt, total - i)) for i in range(0, total, dt)]


# ---------------------------------------------------------------- phase A
def build_phase_a(cfg):
    nc = bacc.Bacc(num_devices=NCORES)
    D = cfg.DIN
    xT = nc.dram_tensor("xT", [D, cfg.NPC], BF16, kind="ExternalInput")
    posT = nc.dram_tensor("posT", [3, cfg.NPC], BF16, kind="ExternalInput")
    h_w1 = nc.dram_tensor("h_w1", [D, D], BF16, kind="ExternalInput")
    h_b1 = nc.dram_tensor("h_b1", [D, 1], F32, kind="ExternalInput")
    h_w2 = nc.dram_tensor("h_w2", [D, 3], BF16, kind="ExternalInput")
    h_b2 = nc.dram_tensor("h_b2", [3, 1], F32, kind="ExternalInput")
    f_w3 = nc.dram_tensor("f_w3", [3, D], BF16, kind="ExternalInput")
    f_wx = nc.dram_tensor("f_wx", [D, D], BF16, kind="ExternalInput")
    f_b = nc.dram_tensor("f_b", [D, 1], F32, kind="ExternalInput")
    uT = nc.dram_tensor("uT", [D, cfg.NPC], BF16, kind="ExternalOutput")
    vT = nc.dram_tensor("vT", [D, cfg.NPC], BF16, kind="ExternalOutput")

    tiles = _dtiles(cfg.NPC, cfg.DT)
    stash = {}
    with tile.TileContext(nc) as tc:
        with (
            tc.tile_pool(name="consts", bufs=1) as cp,
            tc.tile_pool(name="work", bufs=5) as wp,
            tc.tile_pool(name="psum", bufs=2, space="PSUM") as pp,
            tc.tile_pool(name="psumv", bufs=2, space="PSUM") as pv,
        ):
            # HAM warm-up: keep the PE busy while input DMAs land so the
            # real matmuls run at 2.4 GHz from the start.
            warm = cp.tile([D, cfg.DT], BF16)
            nc.vector.memset(warm[:], 0.0)
            wps = pp.tile([D, cfg.DT], F32, tag="psu")
            for _ in range(7):
                nc.tensor.matmul(out=wps[:], lhsT=warm[:, :D], rhs=warm[:],
                                 start=True, stop=True)

            # loads, most-urgent first
            xT_sb = cp.tile([D, cfg.NPC], BF16)
            half = (cfg.NPC // 2) // cfg.DT * cfg.DT or cfg.NPC
            nc.sync.dma_start(out=xT_sb[:, :half], in_=xT[:, :half])
            fwx_sb = cp.tile([D, D], BF16)
            nc.sync.dma_start(out=fwx_sb[:], in_=f_wx[:])
            fw3_sb = cp.tile([3, D], BF16)
            nc.sync.dma_start(out=fw3_sb[:], in_=f_w3[:])
            posT_sb = cp.tile([3, cfg.NPC], BF16)
            nc.sync.dma_start(out=posT_sb[:], in_=posT[:])
            w1_sb = cp.tile([D, D], BF16)
            nc.sync.dma_start(out=w1_sb[:], in_=h_w1[:])
            if half < cfg.NPC:
                nc.sync.dma_start(out=xT_sb[:, half:], in_=xT[:, half:])
            w2_sb = cp.tile([D, 3], BF16)
            nc.sync.dma_start(out=w2_sb[:], in_=h_w2[:])
            b1_sb = cp.tile([D, 1], F32)
            nc.sync.dma_start(out=b1_sb[:], in_=h_b1[:])
            b2_sb = cp.tile([3, 1], F32)
            nc.sync.dma_start(out=b2_sb[:], in_=h_b2[:])
            fb_sb = cp.tile([D, 1], F32)
            nc.sync.dma_start(out=fb_sb[:], in_=f_b[:])

            # u = pos@f_w3 + x@f_wx ; delta = tanh(relu(x@h_w1+h_b1)@h_w2+h_b2)
            # v = (delta - pos)@f_w3 + f_b.  Chunks processed in pairs with
            # matmuls grouped by stationary operand (one LDWEIGHTS per pair);
            # u/v results collect in quarter tiles, stores batched.
            QT = 4 * cfg.DT
            quarters = _dtiles(cfg.NPC, QT)
            ut_q = [cp.tile([D, qw], BF16, name=f"utq{qi}")
                    for qi, (_, qw) in enumerate(quarters)]
            vt_q = [cp.tile([D, qw], BF16, name=f"vtq{qi}")
                    for qi, (_, qw) in enumerate(quarters)]

            def qput(qtiles, off, w):
                return qtiles[off // QT][:, off % QT:off % QT + w]

            def qflush(dram, qtiles, off, w, i, queue):
                qi = off // QT
                if off % QT + w == quarters[qi][1] or i == len(tiles) - 1:
                    q0 = quarters[qi][0]
                    queue.dma_start(out=dram[:, q0:off + w],
                                    in_=qtiles[qi][:, :off + w - q0])

            for p0 in range(0, len(tiles), 2):
                grp = list(enumerate(tiles))[p0:p0 + 2]
                sls = [(i, slice(off, off + w), off, w) for i, (off, w) in grp]
                psu, ps1, ps2, psv, t1s, dms = [], [], [], [], [], []
                for i, sl, off, w in sls:
                    t = pp.tile([D, cfg.DT], F32, tag="psu")
                    nc.tensor.matmul(out=t[:, :w], lhsT=fwx_sb[:],
                                     rhs=xT_sb[:, sl], start=True, stop=False)
                    psu.append(t)
                for (i, sl, off, w), t in zip(sls, psu):
                    nc.tensor.matmul(out=t[:, :w], lhsT=fw3_sb[:],
                                     rhs=posT_sb[:, sl], start=False, stop=True)
                for (i, sl, off, w), t in zip(sls, psu):
                    nc.vector.tensor_copy(out=qput(ut_q, off, w), in_=t[:, :w])
                    qflush(uT, ut_q, off, w, i, nc.gpsimd)
                for i, sl, off, w in sls:
                    t = pp.tile([D, cfg.DT], F32, tag="ps1")
                    nc.tensor.matmul(out=t[:, :w], lhsT=w1_sb[:],
                                     rhs=xT_sb[:, sl], start=True, stop=True)
                    ps1.append(t)
                for (i, sl, off, w), t in zip(sls, ps1):
                    tt = wp.tile([D, cfg.DT], BF16, tag="t1")
                    nc.scalar.activation(out=tt[:, :w], in_=t[:, :w],
                                         func=AF.Relu, bias=b1_sb[:])
                    t1s.append(tt)
                for (i, sl, off, w), tt in zip(sls, t1s):
                    t = pv.tile([3, cfg.DT], F32, tag="ps2")
                    nc.tensor.matmul(out=t[:, :w], lhsT=w2_sb[:], rhs=tt[:, :w],
                                     start=True, stop=True)
                    ps2.append(t)
                for (i, sl, off, w), t in zip(sls, ps2):
                    d = wp.tile([3, cfg.DT], BF16, tag="dm")
                    nc.scalar.activation(out=d[:, :w], in_=t[:, :w],
                                         func=AF.Tanh, bias=b2_sb[:])
                    nc.vector.tensor_tensor(out=d[:, :w], in0=d[:, :w],
                                            in1=posT_sb[:, sl], op=ALU.subtract)
                    dms.append(d)
                for (i, sl, off, w), d in zip(sls, dms):
                    t = pv.tile([D, cfg.DT], F32, tag="psv")
                    nc.tensor.matmul(out=t[:, :w], lhsT=fw3_sb[:], rhs=d[:, :w],
                                     start=True, stop=True)
                    psv.append(t)
                for (i, sl, off, w), t in zip(sls, psv):
                    nc.vector.tensor_scalar_add(out=qput(vt_q, off, w),
                                                in0=t[:, :w], scalar1=fb_sb[:])
                    qflush(vT, vt_q, off, w, i, nc.sync)
    nc.finalize()
    return nc


# ---------------------------------------------------------------- phase B
def build_phase_b(cfg):
    nc = bacc.Bacc(num_devices=NCORES)
    D = cfg.DIN
    TS = cfg.TS
    SCB = cfg.SCB
    NSEC = cfg.NSEC
    SGS = cfg.SG_SECS

    xs = nc.dram_tensor("xs", [128, cfg.NCH, D], BF16, kind="ExternalInput")
    vW = nc.dram_tensor("vW", [128, NSEC, D], BF16, kind="ExternalInput")
    xTb = nc.dram_tensor("xTb", [D, cfg.NPC], BF16, kind="ExternalInput")
    gw1 = nc.dram_tensor("gw1", [D, D], BF16, kind="ExternalInput")
    gb1 = nc.dram_tensor("gb1", [D, 1], F32, kind="ExternalInput")
    gw2 = nc.dram_tensor("gw2", [D, D], BF16, kind="ExternalInput")
    gb2 = nc.dram_tensor("gb2", [D, 1], F32, kind="ExternalInput")
    outT = nc.dram_tensor("outT", [D, cfg.NPC], BF16, kind="ExternalOutput")

    ident = nc.inline_tensor(np.eye(128, dtype=BF), name="ident")

    sg_nch = {}
    for sg in cfg.SGPROC:
        s0, s1 = sg * SGS, min((sg + 1) * SGS, NSEC)
        sg_nch[sg] = int(SCB[s1 - 1] + TS[s1 - 1]) - int(SCB[s0])
    maxc = max(sg_nch.values())

    with tile.TileContext(nc) as tc:
        with (
            tc.tile_pool(name="consts", bufs=1) as cp,
            tc.tile_pool(name="stream", bufs=3) as gp,
            tc.tile_pool(name="cwork", bufs=2) as wp,
            tc.tile_pool(name="psagg", bufs=2, space="PSUM") as pa,
            tc.tile_pool(name="psmlp", bufs=2, space="PSUM") as pm,
        ):
            # vW first, and the first-processed supergroup's sections
            # in their own small DMA so the first adds unblock early
            vW_sb = cp.tile([128, NSEC, D], BF16)
            fs0 = cfg.SGPROC[0] * SGS
            fs1 = min(fs0 + SGS, NSEC)
            nc.sync.dma_start(out=vW_sb[:, fs0:fs1, :], in_=vW[:, fs0:fs1, :])
            if fs0 > 0:
                nc.sync.dma_start(out=vW_sb[:, :fs0, :], in_=vW[:, :fs0, :])
            if fs1 < NSEC:
                nc.sync.dma_start(out=vW_sb[:, fs1:, :], in_=vW[:, fs1:, :])
            ident_sb = cp.tile([128, 128], BF16)
            nc.sync.dma_start(out=ident_sb[:], in_=ident[:])
            gw1_sb = cp.tile([D, D], BF16)
            nc.sync.dma_start(out=gw1_sb[:], in_=gw1[:])
            gw2_sb = cp.tile([D, D], BF16)
            nc.sync.dma_start(out=gw2_sb[:], in_=gw2[:])
            gb1_sb = cp.tile([D, 1], F32)
            nc.sync.dma_start(out=gb1_sb[:], in_=gb1[:])
            gb2_sb = cp.tile([D, 1], F32)
            nc.sync.dma_start(out=gb2_sb[:], in_=gb2[:])
            xtb_sb = cp.tile([D, cfg.NPC], BF16)
            nc.sync.dma_start(out=xtb_sb[:], in_=xTb[:])

            for qi, sg in enumerate(cfg.SGPROC):
                s0 = sg * SGS
                s1 = min(s0 + SGS, NSEC)
                secs = s1 - s0
                c0 = int(SCB[s0])
                nch = sg_nch[sg]

                ue = gp.tile([128, maxc, D], BF16, tag="ue")
                dma_q = nc.gpsimd if qi % 2 == 0 else nc.scalar
                dma_q.dma_start(out=ue[:, :nch, :], in_=xs[:, c0:c0 + nch, :])

                # msg = relu(u + v[dst]) in place, per section (v broadcast
                # across the section's chunks)
                for sv in range(s0, s1):
                    T = int(TS[sv])
                    o = int(SCB[sv]) - c0
                    blk = ue[:, o:o + T, :]
                    nc.vector.tensor_tensor(
                        out=blk, in0=blk,
                        in1=vW_sb[:, sv, None, :].to_broadcast([128, T, D]),
                        op=ALU.add)
                flat = ue[:, :nch, :].rearrange("p c f -> p (c f)")
                nc.vector.tensor_relu(flat, flat)

                # segment-sum into psum [feat, dst]
                ps = pa.tile([D, SGS * cfg.SEC], F32, tag="psagg")
                for j, sv in enumerate(range(s0, s1)):
                    T = int(TS[sv])
                    o = int(SCB[sv]) - c0
                    osl = slice(j * cfg.SEC, (j + 1) * cfg.SEC)
                    for r in range(T):
                        nc.tensor.matmul(out=ps[:, osl], lhsT=ue[:, o + r, :],
                                         rhs=ident_sb[:], start=(r == 0),
                                         stop=(r == T - 1))
                aggt = wp.tile([D, SGS * cfg.SEC], BF16, tag="aggt")
                nc.scalar.activation(out=aggt[:, :secs * cfg.SEC],
                                     in_=ps[:, :secs * cfg.SEC], func=AF.Copy)

                # fused tail: out = x + relu(relu(agg@g_w1+g_b1)@g_w2+g_b2)
                sgw = min(cfg.NPC, s1 * cfg.SEC) - s0 * cfg.SEC
                for toff in range(0, sgw, 512):
                    w = min(512, sgw - toff)
                    n0 = s0 * cfg.SEC + toff
                    nsl = slice(n0, n0 + w)
                    asl = slice(toff, toff + w)
                    ph1 = pm.tile([D, 512], F32, tag="ph1")
                    nc.tensor.matmul(out=ph1[:, :w], lhsT=gw1_sb[:],
                                     rhs=aggt[:, asl], start=True, stop=True)
                    h1 = wp.tile([D, 512], BF16, tag="h1")
                    nc.scalar.activation(out=h1[:, :w], in_=ph1[:, :w],
                                         func=AF.Relu, bias=gb1_sb[:])
                    ph2 = pm.tile([D, 512], F32, tag="ph2")
                    nc.tensor.matmul(out=ph2[:, :w], lhsT=gw2_sb[:],
                                     rhs=h1[:, :w], start=True, stop=True)
                    h2 = wp.tile([D, 512], BF16, tag="h2")
                    nc.scalar.activation(out=h2[:, :w], in_=ph2[:, :w],
                                         func=AF.Relu, bias=gb2_sb[:])
                    ob = wp.tile([D, 512], BF16, tag="ob")
                    nc.gpsimd.tensor_tensor(out=ob[:, :w], in0=h2[:, :w],
                                            in1=xtb_sb[:, nsl], op=ALU.add)
                    nc.sync.dma_start(out=outT[:, nsl], in_=ob[:, :w])
    nc.finalize()
    return nc


# ------------------------------------------------------------ host side
def _preprocess(cfg, edge_index):
    """Sort edges by dst per core; relabel dsts by descending degree;
    column-aligned chunk layout with per-section depth TS (shared across
    cores so one NEFF serves all). Stream chunk order follows the
    supergroup processing order (ramp group first, then largest-first).

    Sets cfg.TS/SCB/NCH/SGPROC/STAGES. Returns per-core dict with:
      idx  [NCH*128] int64  (src node id per slot, -1 pad)
      perm [NPC]     int64  (relabeled id -> original local id)
    """
    src = np.asarray(edge_index[0], dtype=np.int64)
    dst = np.asarray(edge_index[1], dtype=np.int64)
    order = np.argsort(dst, kind="stable")
    src, dst = src[order], dst[order]
    core = dst // cfg.NPC
    bounds = np.searchsorted(core, np.arange(NCORES + 1))

    percore = []
    ts_mat = np.zeros((NCORES, cfg.NSEC), np.int64)
    for c in range(NCORES):
        lo, hi = bounds[c], bounds[c + 1]
        s, d = src[lo:hi], dst[lo:hi] - c * cfg.NPC
        deg = np.bincount(d, minlength=cfg.NPC)
        perm = np.argsort(-deg, kind="stable")
        inv = np.empty(cfg.NPC, np.int64)
        inv[perm] = np.arange(cfg.NPC)
        degpad = np.zeros(cfg.NSEC * cfg.SEC, np.int64)
        degpad[:cfg.NPC] = deg[perm]
        ts_mat[c] = np.maximum(degpad.reshape(cfg.NSEC, cfg.SEC).max(1), 1)
        percore.append((s, d, deg, perm, inv))
    ts = ts_mat.max(0)
    cfg.TS = ts

    # supergroup processing order: smallest first (fast ramp), then the
    # rest largest-first so the big segment-sum tails overlap later work
    sg_ranges = []
    for sg in range(cfg.NSG):
        s0, s1 = sg * cfg.SG_SECS, min((sg + 1) * cfg.SG_SECS, cfg.NSEC)
        sg_ranges.append((sg, s0, s1, int(ts[s0:s1].sum())))
    by_size = sorted(sg_ranges, key=lambda t: t[3])
    if len(by_size) > 2:
        proc = ([by_size[1]] + sorted(by_size[2:], key=lambda t: -t[3])
                + [by_size[0]])
    else:
        proc = by_size
    cfg.SGPROC = [t[0] for t in proc]

    scb = np.zeros(cfg.NSEC, np.int64)
    pos = 0
    stages = []
    st_lo = 0
    for _, s0, s1, n in proc:
        for sv in range(s0, s1):
            scb[sv] = pos
            pos += int(ts[sv])
        if pos - st_lo >= STAGE_CHUNKS or len(stages) == 0:
            stages.append((st_lo, pos))
            st_lo = pos
    if st_lo < pos:
        stages.append((st_lo, pos))
    cfg.SCB = scb
    cfg.NCH = pos
    cfg.STAGES = stages

    out = []
    for c in range(NCORES):
        s, d, deg, perm, inv = percore[c]
        first = np.zeros(cfg.NPC, np.int64)
        np.cumsum(deg[:-1], out=first[1:])
        rank = np.arange(len(d)) - first[d]
        k = inv[d]
        chunk = scb[k >> 7] + rank
        slot = chunk * 128 + (k & 127)
        idx = np.full(cfg.NCH * 128, -1, np.int64)
        idx[slot] = s
        out.append({"idx": idx, "perm": perm})
    return out


def _expand_stream(tbl, idx, nch):
    """Gather rows of tbl by idx (PAD_U row for idx<0) -> [128, nch, D]."""
    rows = np.full((len(idx), tbl.shape[1]), PAD_U, dtype=tbl.dtype)
    valid = idx >= 0
    rows[valid] = tbl[idx[valid]]
    return np.ascontiguousarray(
        rows.reshape(nch, 128, -1).transpose(1, 0, 2))


def run(cfg, inputs, trace=False):
    """Full pipeline. inputs: dict as from setup_inputs (numpy)."""
    x = np.asarray(inputs["x"], np.float32)
    pos = np.asarray(inputs["pos"], np.float32)
    edata = _preprocess(cfg, np.asarray(inputs["edge_index"]))

    h_w1 = np.asarray(inputs["h_w1"], np.float32)
    h_b1 = np.asarray(inputs["h_b1"], np.float32)
    h_w2 = np.asarray(inputs["h_w2"], np.float32)
    h_b2 = np.asarray(inputs["h_b2"], np.float32)
    f_w = np.asarray(inputs["f_w"], np.float32)
    f_b = np.asarray(inputs["f_b"], np.float32)
    g_w1 = np.asarray(inputs["g_w1"], np.float32)
    g_b1 = np.asarray(inputs["g_b1"], np.float32)
    g_w2 = np.asarray(inputs["g_w2"], np.float32)
    g_b2 = np.asarray(inputs["g_b2"], np.float32)

    nc_a = build_phase_a(cfg)
    in_a = []
    for c in range(NCORES):
        sl = slice(c * cfg.NPC, (c + 1) * cfg.NPC)
        in_a.append({
            "xT": np.ascontiguousarray(x[sl].T.astype(BF)),
            "posT": np.ascontiguousarray(pos[sl].T.astype(BF)),
            "h_w1": h_w1.astype(BF), "h_b1": h_b1[:, None],
            "h_w2": h_w2.astype(BF), "h_b2": h_b2[:, None],
            "f_w3": f_w[:3].astype(BF), "f_wx": f_w[3:].astype(BF),
            "f_b": f_b[:, None],
        })
    res_a = run_bass_kernel_spmd(nc_a, in_a, core_ids=list(range(NCORES)),
                                 trace=trace)
    # u table node-major over ALL nodes; v tables per-core node-major
    u_nm = np.concatenate(
        [np.ascontiguousarray(np.asarray(r["uT"]).T) for r in res_a.results],
        axis=0)
    v_nms = [np.ascontiguousarray(np.asarray(r["vT"]).T) for r in res_a.results]

    nc_b = build_phase_b(cfg)
    in_b = []
    for c in range(NCORES):
        sl = slice(c * cfg.NPC, (c + 1) * cfg.NPC)
        ed = edata[c]
        perm = ed["perm"]
        v_nm = v_nms[c]
        # vW [128, NSEC, D]: vW[p, s] = v[perm[s*128+p]] (zero-pad past NPC)
        vpad = np.zeros((cfg.NSEC * cfg.SEC, cfg.DIN), dtype=v_nm.dtype)
        vpad[:cfg.NPC] = v_nm[perm]
        vW = np.ascontiguousarray(
            vpad.reshape(cfg.NSEC, 128, cfg.DIN).transpose(1, 0, 2))
        xl = x[sl].astype(BF)
        in_b.append({
            "xs": _expand_stream(u_nm, ed["idx"], cfg.NCH),
            "vW": vW,
            "xTb": np.ascontiguousarray(xl[perm].T),
            "gw1": g_w1.astype(BF), "gb1": g_b1[:, None],
            "gw2": g_w2.astype(BF), "gb2": g_b2[:, None],
        })
    res_b = run_bass_kernel_spmd(nc_b, in_b, core_ids=list(range(NCORES)),
                                 trace=trace)
    out = np.empty((cfg.N, cfg.DIN), np.float32)
    for c in range(NCORES):
        rows = np.asarray(res_b.results[c]["outT"]).T.astype(np.float32)
        blk = out[c * cfg.NPC:(c + 1) * cfg.NPC]
        blk[edata[c]["perm"]] = rows
    return out, (res_a, res_b)


DEFAULT_CFG = Cfg(n=50000, e=500000, din=128)


def kernel(**inputs):
    out, _ = run(DEFAULT_CFG, inputs)
    return out.astype(np.float32)
